# revision 24
# baseline (speedup 1.0000x reference)
"""Trainium2 Bass kernel for Swin-style window attention with Euclidean-distance
scores (nn_Attention_2_59373627899920).

Math per (b, h):
    z[j, i]  = q2[i] + k2[j] - 2 * sum_d q[i,d] k[j,d]        (bf16 matmul, K=34 augmented)
    d'[j, i] = sqrt(z)/sqrt(2)                                 (ACT Sqrt(scale=0.5), f16)
    E[j, i]  = exp(sqrt(z) + bias + mask)  via the Schraudolph bit trick:
               bf16_bits(E) = round(261.1294*d' + cs[j,i])  with
               cs = round(184.6629*(bias+mask) + 16249) precomputed on host (u16).
               One fused DVE (or GPSIMD) instruction per 2-b group replaces the
               separate bias add AND the ACT Exp pass (~1.8% rms multiplicative
               noise on E; softmax-level output error ~1.5e-2 < the 2e-2 gate).
    pv[i, c] = sum_j E[j, i] * v_aug[j, c]   c in 0..32        (PE, E stationary; c=32 is
                                                               ones column -> denominator)
    output: pv (numerators + denominator) copied PSUM->SBUF (ACT/DVE) and DMA'd out;
    the final divide x = pv[:, :32] / pv[:, 32] happens on the host after gather.

Scores are built TRANSPOSED (j on partitions) so the softmax reduction is folded
into the PV matmul via the ones column, and no row-max subtraction is needed
(logits are bounded, exp fits bf16).

The ACT engine runs ONLY Sqrt (+ half the pv copies): one table load for the
whole kernel. Phase E of chunk k is emitted interleaved into phase S of chunk
k+1 so the PE's PV matmuls smear between z productions and every engine stays
fed. Sharding: data-parallel over B_ = 256: core c owns windows 8c..8c+7 x 4
batches (32 windows*batch each).
"""

import os
import sys
from contextlib import ExitStack

import numpy as np

sys.path.insert(0, "/opt/trn_rl_repo")

import ml_dtypes  # noqa: E402

import concourse.bacc as bacc  # noqa: E402
import concourse.mybir as mybir  # noqa: E402
import concourse.tile as tile  # noqa: E402
from concourse.dve_ops import (  # noqa: E402
    CUSTOM_DVE_SPECS,
    OPS,
    _SUB_OPCODE_FOR_NAME,
    DveOp,
)
from concourse.dve_spec import C0 as SC0  # noqa: E402
from concourse.dve_spec import Spec, Src0, Src1, _has_src1, lower  # noqa: E402
from concourse.dve_uop import DveOpSpec  # noqa: E402


def _register_dve_op(name, spec):
    """Register a kernel-local custom DVE op in the module-level registries
    used by codegen (sub-opcode map), table-gen (OPS) and CoreSim (SPECS)."""
    for op in OPS:
        if op.name == name:
            return op
    row = max(_SUB_OPCODE_FOR_NAME.values()) + 1
    assert row < 0x20, "byte-36 row field is 5 bits"
    _SUB_OPCODE_FOR_NAME[name] = row
    uops = lower(spec, ver="v3")
    sha = DveOpSpec(name=name, opcode=row, uops=uops, rd1_en=_has_src1(spec)).sha(
        "v3"
    )
    op = DveOp(name, spec, subdim=False, uops_sha={"v3": sha})
    OPS.append(op)
    CUSTOM_DVE_SPECS[name] = spec
    return op


# Fused bias-add + exp via the Schraudolph bit trick, written straight into the
# bf16 E tile (viewed as u16): bits = in0*c0 + in1, in1 = host-precomputed
# cs = 184.6629*(bias+mask) + 16249 (u16 ints), c0 = 128*log2(e)*sqrt(2).
SCHRAUD_EXP_ANT = _register_dve_op(
    "SCHRAUD_EXP_ANT",
    Spec(
        body=Src0 * SC0 + Src1,
        reference=lambda in0, in1, c0, c1, imm2: (
            in0.astype(np.float32) * c0
            + in1.astype(np.float32).reshape(in0.shape)
        ),
    ),
)

F32 = mybir.dt.float32
BF16 = mybir.dt.bfloat16
F16 = mybir.dt.float16
U16 = mybir.dt.uint16
SQRT2 = float(np.sqrt(2.0))
LOG2E = 1.4426950408889634
SCHRAUD_C1 = 128.0 * LOG2E * SQRT2      # multiplies d' = sqrt(z)/sqrt(2)
SCHRAUD_C0 = 16249.0                    # 16256 - 7.5 (rms-optimal) + 0.5 (trunc)

NH, HD, N, NW, B_ = 6, 32, 256, 64, 256
NCORES = 8
NB = B_ // NCORES          # 32 windows*batch per core
NWC = NW // NCORES         # 8 windows per core
NBATCH = B_ // NW          # 4 batches
CB = 8                     # b's per chunk (2 windows x 4 batches)
NCHUNK = NB // CB
DA = HD + 2                # augmented contraction dim: [k; k2; 1] . [-2q; 1; q2]
VC = HD + 1                # v columns per head incl. ones column

# --- engine-routing knobs --------------------------------------------------
# per chunk there are 4 groups of 2 b's; groups in SCHRAUD_POOL_G run the
# fused exp-add on GPSIMD tensor_add, the rest on DVE tensor_add (2x mode:
# all operands 2-byte SBUF). The Schraudolph multiply is folded into the ACT
# Sqrt's input scale, so the fused exp is a plain add of d'' and cs.
SCHRAUD_POOL_G = frozenset()
# pv PSUM->SBUF copy: bh % 2 == COPY_ACT_PARITY -> ACT Copy, else DVE
# (2 = never on ACT).
COPY_ACT_PARITY = 2


def build_nc():
    """Build the single-core SPMD graph (all 8 cores run the same program)."""
    nc = bacc.Bacc("TRN2", target_bir_lowering=False, debug=False, num_devices=NCORES)

    ab = nc.declare_dram_parameter("ab", [NB, DA, 2 * NH * N], BF16, isOutput=False)
    cs = nc.declare_dram_parameter("cs", [NWC, 128, 2 * NH * N], U16, isOutput=False)
    vp = nc.declare_dram_parameter("vp", [128, 2 * NB * NH * VC], BF16, isOutput=False)
    po = nc.declare_dram_parameter("po", [NB, 128, 2 * NH * VC], F32, isOutput=True)

    SQRT = mybir.ActivationFunctionType.Sqrt
    COPY = mybir.ActivationFunctionType.Copy
    MULT = mybir.AluOpType.mult
    ADD = mybir.AluOpType.add

    with tile.TileContext(nc) as tc, ExitStack() as ctx:
        abp = ctx.enter_context(tc.tile_pool(name="abp", bufs=3))
        csp = ctx.enter_context(tc.tile_pool(name="csp", bufs=4))
        vpp = ctx.enter_context(tc.tile_pool(name="vpp", bufs=1))
        dap = ctx.enter_context(tc.tile_pool(name="dap", bufs=2))
        ep = ctx.enter_context(tc.tile_pool(name="ep", bufs=2))
        psp = ctx.enter_context(tc.tile_pool(name="psp", bufs=3))
        zpp = ctx.enter_context(tc.tile_pool(name="zpp", bufs=2, space="PSUM"))
        pvp = ctx.enter_context(tc.tile_pool(name="pvp", bufs=2, space="PSUM"))

        # small epsilon bias for Sqrt (guards z ~ -1e-5 rounding negatives);
        # scaled to match the Schraudolph input scale below
        epsb = vpp.tile([128, 1], F32)
        nc.vector.memset(epsb[:, :], 1e-4 * 0.5 * SCHRAUD_C1 * SCHRAUD_C1)

        # v (+ ones col) for the whole core, loaded once (at l==2 below: after
        # the first ab loads, before the first PV matmuls need it)
        vpt = vpp.tile([128, 2 * NB * NH * VC], BF16)

        # pending phase-E half-steps of the previous chunk, emitted interleaved
        # into the current chunk's phase S (one half-step per b so PV matmuls
        # smear finely between z productions and the PE never idles long)
        pend = []  # list of closures

        def emit_exp(da_t, cs_t, g, E):
            """fused Schraudolph exp-add for the 2-b group g."""
            e_u16 = E[:, :].bitcast(U16).rearrange(
                "p (b c) -> p b c", b=2, c=NH * 2 * N
            )
            src = da_t[:, 2 * g * NH * 2 * N : (2 * g + 2) * NH * 2 * N].rearrange(
                "p (b c) -> p b c", b=2, c=NH * 2 * N
            )
            cs_b = cs_t[:, :].unsqueeze(1).broadcast_to([128, 2, NH * 2 * N])
            if g in SCHRAUD_POOL_G:
                nc.gpsimd.tensor_add(e_u16, src, cs_b)
            else:
                nc.vector.tensor_add(e_u16, src, cs_b)

        def emit_b_tail(E, eo, l, bh):
            """PV matmuls + PSUM->SBUF copy + store for one b."""
            pv = pvp.tile([128, 2 * NH * VC], F32)
            for h in range(NH):
                for ih in range(2):
                    for jh in range(2):
                        nc.tensor.matmul(
                            pv[:, ih * NH * VC + h * VC : ih * NH * VC + (h + 1) * VC],
                            E[:, eo + (h * 2 + jh) * N + ih * 128 : eo + (h * 2 + jh) * N + ih * 128 + 128],
                            vpt[:, (jh * NB + l) * NH * VC + h * VC : (jh * NB + l) * NH * VC + (h + 1) * VC],
                            start=(jh == 0),
                            stop=(jh == 1),
                        )
            pvs = psp.tile([128, 2 * NH * VC], F32)
            if bh % 2 == COPY_ACT_PARITY:
                nc.scalar.activation(pvs[:, :], pv[:, :], COPY)
            else:
                nc.vector.tensor_copy(out=pvs[:, :], in_=pv[:, :])
            nc.gpsimd.dma_start(out=po.ap()[l], in_=pvs[:, :])

        def queue_chunk_part(da_t, cs_tiles, chunk0, g):
            cell = {}

            def half0(g=g, cell=cell, da_t=da_t, cs_t=cs_tiles[g]):
                E = ep.tile([128, 2 * NH * 2 * N], BF16)
                cell["E"] = E
                emit_exp(da_t, cs_t, g, E)
                emit_b_tail(E, 0, chunk0 + 2 * g, 2 * g)

            def half1(g=g, cell=cell):
                emit_b_tail(cell["E"], NH * 2 * N, chunk0 + 2 * g + 1, 2 * g + 1)

            pend.append(half0)
            pend.append(half1)

        def queue_chunk(da_t, cs_tiles, chunk0):
            for g in range(CB // 2):
                queue_chunk_part(da_t, cs_tiles, chunk0, g)

        cst = None
        for k in range(NCHUNK):
            chunk0 = k * CB
            da = dap.tile([128, CB * NH * 2 * N], F16)
            da_v = da[:, :].rearrange(
                "p (b h jh i) -> p b h jh i", b=CB, h=NH, jh=2, i=N
            )
            cs_of_group = []
            for bh in range(CB):
                l = chunk0 + bh
                w_l = l // NBATCH
                abt = abp.tile([DA, 2 * NH * N], BF16)
                # all loads via SWDGE (gpsimd): HWDGE (sync) stripes each DMA
                # over a single engine pair (E64/65) and becomes the critical
                # path; SWDGE spreads descriptors over all 16 engines
                nc.gpsimd.dma_start(out=abt[:, :], in_=ab.ap()[l])
                if l % NBATCH == 0:
                    cst = csp.tile([128, 2 * NH * N], U16)
                    nc.gpsimd.dma_start(out=cst[:, :], in_=cs.ap()[w_l])
                if l == 2:
                    nc.gpsimd.dma_start(out=vpt[:, :], in_=vp.ap())
                if bh % 2 == 0:
                    cs_of_group.append(cst)
                for jh in range(2):
                    z = zpp.tile([128, NH * N], F32)
                    for h in range(NH):
                        lhsT = abt[:, h * N + jh * 128 : h * N + jh * 128 + 128]
                        rhs = abt[:, (NH + h) * N : (NH + h) * N + N]
                        nc.tensor.matmul(
                            z[:, h * N : (h + 1) * N],
                            lhsT,
                            rhs,
                            start=True,
                            stop=True,
                        )
                    # d'' = C1*sqrt(z)/sqrt(2) = sqrt(z*C1^2/2): the Schraudolph
                    # multiply folded into the ACT Sqrt input scale, f16.
                    # Two half-tile activations so the first can start while
                    # the PE is still filling the tile's last 3 heads.
                    for hh in range(2):
                        nc.scalar.activation(
                            da_v[:, bh, 3 * hh : 3 * hh + 3, jh, :],
                            z[:, 3 * hh * N : (3 * hh + 3) * N],
                            SQRT,
                            bias=epsb[:, :],
                            scale=0.5 * SCHRAUD_C1 * SCHRAUD_C1,
                        )
                # after each b, emit one pending half-step of the previous
                # chunk so PV matmuls smear finely between z productions
                if pend:
                    pend.pop(0)()
            queue_chunk(da, cs_of_group, chunk0)
        while pend:
            pend.pop(0)()

    nc.compile()
    return nc


def prep_inputs(q, k, v, table, mask, index):
    """Host-side sharding/layout prep. Returns in_maps for the 8 cores."""
    q = np.asarray(q, np.float32)
    k = np.asarray(k, np.float32)
    v = np.asarray(v, np.float32)
    table = np.asarray(table, np.float32)
    mask = np.asarray(mask, np.float32)
    index = np.asarray(index)

    q2 = (q * q).sum(-1)  # [B_, NH, N]
    k2 = (k * k).sum(-1)

    # ab[l, 0] = [kT; k2; 1]; ab[l, 1] = [-2 qT; 1; q2]   (both [NH, 34, N])
    ones = np.ones((B_, NH, 1, N), np.float32)
    ab_k = np.concatenate(
        [k.transpose(0, 1, 3, 2), k2[:, :, None, :], ones], axis=2
    )  # [B_, NH, 34, N]
    ab_q = np.concatenate(
        [-2.0 * q.transpose(0, 1, 3, 2), ones, q2[:, :, None, :]], axis=2
    )
    ab_full = np.stack([ab_k, ab_q], axis=1)  # [B_, 2, NH, 34, N]
    ab_full = (
        np.ascontiguousarray(ab_full.transpose(0, 3, 1, 2, 4))
        .reshape(B_, DA, 2 * NH * N)
        .astype(ml_dtypes.bfloat16)
    )

    # cs[w, jj, h, jh, i] = round(184.66*(bias[h,i,j] + mask[w,i,j]) + C0), u16
    bias = table[index].reshape(N, N, NH)  # [i, j, h]
    biasT = np.ascontiguousarray(bias.transpose(2, 1, 0))  # [h, j, i]
    maskT = mask.transpose(0, 2, 1)  # [w, j, i]
    cfull = np.rint(
        (biasT[None] + maskT[:, None]) * np.float32(128.0 * LOG2E)
        + np.float32(SCHRAUD_C0)
    ).astype(np.uint16)
    cfull = np.ascontiguousarray(
        cfull.reshape(NW, NH, 2, 128, N).transpose(0, 3, 1, 2, 4)
    ).reshape(NW, 128, 2 * NH * N)

    # vp[jh, jj, l, h*33+c]
    v_aug = np.concatenate([v, np.ones((B_, NH, N, 1), np.float32)], axis=-1)

    in_maps = []
    bg_lists = []
    for c in range(NCORES):
        bg = np.array(
            [b * NW + 8 * c + wl for wl in range(NWC) for b in range(NBATCH)]
        )
        bg_lists.append(bg)
        va = v_aug[bg]  # [32, NH, N, 33]
        vpc = np.ascontiguousarray(
            va.transpose(2, 0, 1, 3)
            .reshape(2, 128, NB, NH * VC)
            .transpose(1, 0, 2, 3)
            .reshape(128, 2 * NB * NH * VC)
        ).astype(ml_dtypes.bfloat16)
        in_maps.append(
            {
                "ab": np.ascontiguousarray(ab_full[bg]),
                "cs": np.ascontiguousarray(cfull[8 * c : 8 * c + 8]),
                "vp": vpc,
            }
        )
    return in_maps, bg_lists


def postprocess(po_arr):
    """Normalize one core's pv output: [NB, 128, 2*NH*VC] -> [NB, N, NH*HD]."""
    v = np.asarray(po_arr, np.float32).reshape(NB, 128, 2, NH, VC)
    x = v[..., :HD] / v[..., HD:HD + 1]        # [NB, p, ih, NH, HD]
    return np.ascontiguousarray(x.transpose(0, 2, 1, 3, 4)).reshape(NB, N, NH * HD)


_NC_CACHE = {}


def get_nc():
    if "nc" not in _NC_CACHE:
        _NC_CACHE["nc"] = build_nc()
    return _NC_CACHE["nc"]


def kernel(q, k, v, table, mask, index):
    from concourse.bass_utils import run_bass_kernel_spmd

    in_maps, bg_lists = prep_inputs(q, k, v, table, mask, index)
    nc = get_nc()
    res = run_bass_kernel_spmd(nc, in_maps, core_ids=list(range(NCORES)))
    out = np.empty((B_, N, NH * HD), np.float32)
    for c in range(NCORES):
        out[bg_lists[c]] = postprocess(res.results[c]["po"])
    return out


if __name__ == "__main__":
    nc = build_nc()
    print("build + compile OK")


# revision 27
# speedup vs baseline: 1.2855x; 1.2855x over previous
"""Trainium2 Bass kernel for Swin-style window attention with Euclidean-distance
scores (nn_Attention_2_59373627899920).

Math per (b, h):
    z[j, i]  = q2[i] + k2[j] - 2 * sum_d q[i,d] k[j,d]        (bf16 matmul, K=34 augmented)
    d'[j, i] = sqrt(z)/sqrt(2)                                 (ACT Sqrt(scale=0.5), f16)
    E[j, i]  = exp(sqrt(z) + bias + mask)  via the Schraudolph bit trick:
               bf16_bits(E) = round(261.1294*d' + cs[j,i])  with
               cs = round(184.6629*(bias+mask) + 16249) precomputed on host (u16).
               One fused DVE (or GPSIMD) instruction per 2-b group replaces the
               separate bias add AND the ACT Exp pass (~1.8% rms multiplicative
               noise on E; softmax-level output error ~1.5e-2 < the 2e-2 gate).
    pv[i, c] = sum_j E[j, i] * v_aug[j, c]   c in 0..32        (PE, E stationary; c=32 is
                                                               ones column -> denominator)
    output: pv (numerators + denominator) copied PSUM->SBUF (ACT/DVE) and DMA'd out;
    the final divide x = pv[:, :32] / pv[:, 32] happens on the host after gather.

Scores are built TRANSPOSED (j on partitions) so the softmax reduction is folded
into the PV matmul via the ones column, and no row-max subtraction is needed
(logits are bounded, exp fits bf16).

The ACT engine runs ONLY Sqrt (+ half the pv copies): one table load for the
whole kernel. Phase E of chunk k is emitted interleaved into phase S of chunk
k+1 so the PE's PV matmuls smear between z productions and every engine stays
fed. Sharding: data-parallel over B_ = 256: core c owns windows 8c..8c+7 x 4
batches (32 windows*batch each).
"""

import os
import sys
from contextlib import ExitStack

import numpy as np

sys.path.insert(0, "/opt/trn_rl_repo")

import ml_dtypes  # noqa: E402

import concourse.bacc as bacc  # noqa: E402
import concourse.mybir as mybir  # noqa: E402
import concourse.tile as tile  # noqa: E402
from concourse.dve_ops import (  # noqa: E402
    CUSTOM_DVE_SPECS,
    OPS,
    _SUB_OPCODE_FOR_NAME,
    DveOp,
)
from concourse.dve_spec import C0 as SC0  # noqa: E402
from concourse.dve_spec import Spec, Src0, Src1, _has_src1, lower  # noqa: E402
from concourse.dve_uop import DveOpSpec  # noqa: E402


def _register_dve_op(name, spec):
    """Register a kernel-local custom DVE op in the module-level registries
    used by codegen (sub-opcode map), table-gen (OPS) and CoreSim (SPECS)."""
    for op in OPS:
        if op.name == name:
            return op
    row = max(_SUB_OPCODE_FOR_NAME.values()) + 1
    assert row < 0x20, "byte-36 row field is 5 bits"
    _SUB_OPCODE_FOR_NAME[name] = row
    uops = lower(spec, ver="v3")
    sha = DveOpSpec(name=name, opcode=row, uops=uops, rd1_en=_has_src1(spec)).sha(
        "v3"
    )
    op = DveOp(name, spec, subdim=False, uops_sha={"v3": sha})
    OPS.append(op)
    CUSTOM_DVE_SPECS[name] = spec
    return op


# Fused bias-add + exp via the Schraudolph bit trick, written straight into the
# bf16 E tile (viewed as u16): bits = in0*c0 + in1, in1 = host-precomputed
# cs = 184.6629*(bias+mask) + 16249 (u16 ints), c0 = 128*log2(e)*sqrt(2).
SCHRAUD_EXP_ANT = _register_dve_op(
    "SCHRAUD_EXP_ANT",
    Spec(
        body=Src0 * SC0 + Src1,
        reference=lambda in0, in1, c0, c1, imm2: (
            in0.astype(np.float32) * c0
            + in1.astype(np.float32).reshape(in0.shape)
        ),
    ),
)

F32 = mybir.dt.float32
BF16 = mybir.dt.bfloat16
F16 = mybir.dt.float16
U16 = mybir.dt.uint16
SQRT2 = float(np.sqrt(2.0))
LOG2E = 1.4426950408889634
SCHRAUD_C1 = 128.0 * LOG2E * SQRT2      # multiplies d' = sqrt(z)/sqrt(2)
SCHRAUD_C0 = 16249.0                    # 16256 - 7.5 (rms-optimal) + 0.5 (trunc)

NH, HD, N, NW, B_ = 6, 32, 256, 64, 256
NCORES = 8
NB = B_ // NCORES          # 32 windows*batch per core
NWC = NW // NCORES         # 8 windows per core
NBATCH = B_ // NW          # 4 batches
CB = 8                     # b's per chunk (2 windows x 4 batches)
NCHUNK = NB // CB
DA = HD + 2                # augmented contraction dim: [k; k2; 1] . [-2q; 1; q2]
VC = HD + 1                # v columns per head incl. ones column

# --- engine-routing knobs --------------------------------------------------
# per chunk there are 4 groups of 2 b's; groups in SCHRAUD_POOL_G run the
# fused exp-add on GPSIMD tensor_add, the rest on DVE tensor_add (2x mode:
# all operands 2-byte SBUF). The Schraudolph multiply is folded into the ACT
# Sqrt's input scale, so the fused exp is a plain add of d'' and cs.
SCHRAUD_POOL_G = frozenset()
# pv PSUM->SBUF copy: bh % 2 == COPY_ACT_PARITY -> ACT Copy, else DVE
# (2 = never on ACT).
COPY_ACT_PARITY = 2


def build_nc():
    """Build the single-core SPMD graph (all 8 cores run the same program)."""
    nc = bacc.Bacc("TRN2", target_bir_lowering=False, debug=False, num_devices=NCORES)

    ab = nc.declare_dram_parameter("ab", [NB, DA, 2 * NH * N], BF16, isOutput=False)
    cs = nc.declare_dram_parameter("cs", [NWC, 128, 2 * NH * N], U16, isOutput=False)
    vp = nc.declare_dram_parameter("vp", [128, 2 * NB * NH * VC], BF16, isOutput=False)
    po = nc.declare_dram_parameter("po", [NB, 128, 2 * NH * VC], F32, isOutput=True)

    SQRT = mybir.ActivationFunctionType.Sqrt
    COPY = mybir.ActivationFunctionType.Copy
    MULT = mybir.AluOpType.mult
    ADD = mybir.AluOpType.add

    with tile.TileContext(nc) as tc, ExitStack() as ctx:
        abp = ctx.enter_context(tc.tile_pool(name="abp", bufs=3))
        csp = ctx.enter_context(tc.tile_pool(name="csp", bufs=4))
        vpp = ctx.enter_context(tc.tile_pool(name="vpp", bufs=1))
        dap = ctx.enter_context(tc.tile_pool(name="dap", bufs=2))
        ep = ctx.enter_context(tc.tile_pool(name="ep", bufs=2))
        psp = ctx.enter_context(tc.tile_pool(name="psp", bufs=3))
        zpp = ctx.enter_context(tc.tile_pool(name="zpp", bufs=2, space="PSUM"))
        pvp = ctx.enter_context(tc.tile_pool(name="pvp", bufs=2, space="PSUM"))

        # small epsilon bias for Sqrt (guards z ~ -1e-5 rounding negatives);
        # scaled to match the Schraudolph input scale below
        epsb = vpp.tile([128, 1], F32)
        nc.vector.memset(epsb[:, :], 1e-4 * 0.5 * SCHRAUD_C1 * SCHRAUD_C1)

        # v (+ ones col) for the whole core, loaded once (at l==2 below: after
        # the first ab loads, before the first PV matmuls need it)
        vpt = vpp.tile([128, 2 * NB * NH * VC], BF16)

        # pending phase-E half-steps of the previous chunk, emitted interleaved
        # into the current chunk's phase S (one half-step per b so PV matmuls
        # smear finely between z productions and the PE never idles long)
        pend = []  # list of closures

        def emit_exp(da_t, cs_t, g, E):
            """fused Schraudolph exp-add for the 2-b group g."""
            e_u16 = E[:, :].bitcast(U16).rearrange(
                "p (b c) -> p b c", b=2, c=NH * 2 * N
            )
            src = da_t[:, 2 * g * NH * 2 * N : (2 * g + 2) * NH * 2 * N].rearrange(
                "p (b c) -> p b c", b=2, c=NH * 2 * N
            )
            cs_b = cs_t[:, :].unsqueeze(1).broadcast_to([128, 2, NH * 2 * N])
            if g in SCHRAUD_POOL_G:
                nc.gpsimd.tensor_add(e_u16, src, cs_b)
            else:
                nc.vector.tensor_add(e_u16, src, cs_b)

        def emit_b_tail(E, eo, l, bh, copy_act=None):
            """PV matmuls + PSUM->SBUF copy + store for one b."""
            pv = pvp.tile([128, 2 * NH * VC], F32)
            for h in range(NH):
                for ih in range(2):
                    for jh in range(2):
                        nc.tensor.matmul(
                            pv[:, ih * NH * VC + h * VC : ih * NH * VC + (h + 1) * VC],
                            E[:, eo + (h * 2 + jh) * N + ih * 128 : eo + (h * 2 + jh) * N + ih * 128 + 128],
                            vpt[:, (jh * NB + l) * NH * VC + h * VC : (jh * NB + l) * NH * VC + (h + 1) * VC],
                            start=(jh == 0),
                            stop=(jh == 1),
                        )
            pvs = psp.tile([128, 2 * NH * VC], F32)
            if copy_act is None:
                copy_act = bh % 2 == COPY_ACT_PARITY
            if copy_act:
                nc.scalar.activation(pvs[:, :], pv[:, :], COPY)
            else:
                nc.vector.tensor_copy(out=pvs[:, :], in_=pv[:, :])
            nc.gpsimd.dma_start(out=po.ap()[l], in_=pvs[:, :])

        def queue_chunk_part(da_t, cs_tiles, chunk0, g):
            cell = {}

            def half0(g=g, cell=cell, da_t=da_t, cs_t=cs_tiles[g]):
                E = ep.tile([128, 2 * NH * 2 * N], BF16)
                cell["E"] = E
                emit_exp(da_t, cs_t, g, E)
                emit_b_tail(E, 0, chunk0 + 2 * g, 2 * g)

            def half1(g=g, cell=cell):
                emit_b_tail(cell["E"], NH * 2 * N, chunk0 + 2 * g + 1, 2 * g + 1)

            pend.append(half0)
            pend.append(half1)

        def queue_chunk(da_t, cs_tiles, chunk0):
            for g in range(CB // 2):
                queue_chunk_part(da_t, cs_tiles, chunk0, g)

        cst = None
        for k in range(NCHUNK):
            chunk0 = k * CB
            da = dap.tile([128, CB * NH * 2 * N], F16)
            da_v = da[:, :].rearrange(
                "p (b h jh i) -> p b h jh i", b=CB, h=NH, jh=2, i=N
            )
            cs_of_group = []
            for bh in range(CB):
                l = chunk0 + bh
                w_l = l // NBATCH
                abt = abp.tile([DA, 2 * NH * N], BF16)
                # all loads via SWDGE (gpsimd): HWDGE (sync) stripes each DMA
                # over a single engine pair (E64/65) and becomes the critical
                # path; SWDGE spreads descriptors over all 16 engines
                nc.gpsimd.dma_start(out=abt[:, :], in_=ab.ap()[l])
                if l % NBATCH == 0:
                    cst = csp.tile([128, 2 * NH * N], U16)
                    nc.gpsimd.dma_start(out=cst[:, :], in_=cs.ap()[w_l])
                if l == 2:
                    nc.gpsimd.dma_start(out=vpt[:, :], in_=vp.ap())
                if bh % 2 == 0:
                    cs_of_group.append(cst)
                for jh in range(2):
                    z = zpp.tile([128, NH * N], F32)
                    for h in range(NH):
                        lhsT = abt[:, h * N + jh * 128 : h * N + jh * 128 + 128]
                        rhs = abt[:, (NH + h) * N : (NH + h) * N + N]
                        nc.tensor.matmul(
                            z[:, h * N : (h + 1) * N],
                            lhsT,
                            rhs,
                            start=True,
                            stop=True,
                        )
                    # d'' = C1*sqrt(z)/sqrt(2) = sqrt(z*C1^2/2): the Schraudolph
                    # multiply folded into the ACT Sqrt input scale, f16
                    nc.scalar.activation(
                        da_v[:, bh, :, jh, :],
                        z[:, :],
                        SQRT,
                        bias=epsb[:, :],
                        scale=0.5 * SCHRAUD_C1 * SCHRAUD_C1,
                    )
                # after each b, emit one pending half-step of the previous
                # chunk so PV matmuls smear finely between z productions
                if pend:
                    pend.pop(0)()
            if k < NCHUNK - 1:
                queue_chunk(da, cs_of_group, chunk0)
            else:
                last_da, last_cs, last_c0 = da, cs_of_group, chunk0
        while pend:
            pend.pop(0)()
        # tail chunk: per-b exps (finer pipeline), direct cs (no broadcast),
        # copies on the now-idle ACT engine
        for bh in range(CB):
            E = ep.tile([128, NH * 2 * N], BF16)
            nc.vector.tensor_add(
                E[:, :].bitcast(U16),
                last_da[:, bh * NH * 2 * N : (bh + 1) * NH * 2 * N],
                last_cs[bh // 2][:, :],
            )
            emit_b_tail(E, 0, last_c0 + bh, bh, copy_act=True)

    nc.compile()
    return nc


def prep_inputs(q, k, v, table, mask, index):
    """Host-side sharding/layout prep. Returns in_maps for the 8 cores."""
    q = np.asarray(q, np.float32)
    k = np.asarray(k, np.float32)
    v = np.asarray(v, np.float32)
    table = np.asarray(table, np.float32)
    mask = np.asarray(mask, np.float32)
    index = np.asarray(index)

    q2 = (q * q).sum(-1)  # [B_, NH, N]
    k2 = (k * k).sum(-1)

    # ab[l, 0] = [kT; k2; 1]; ab[l, 1] = [-2 qT; 1; q2]   (both [NH, 34, N])
    ones = np.ones((B_, NH, 1, N), np.float32)
    ab_k = np.concatenate(
        [k.transpose(0, 1, 3, 2), k2[:, :, None, :], ones], axis=2
    )  # [B_, NH, 34, N]
    ab_q = np.concatenate(
        [-2.0 * q.transpose(0, 1, 3, 2), ones, q2[:, :, None, :]], axis=2
    )
    ab_full = np.stack([ab_k, ab_q], axis=1)  # [B_, 2, NH, 34, N]
    ab_full = (
        np.ascontiguousarray(ab_full.transpose(0, 3, 1, 2, 4))
        .reshape(B_, DA, 2 * NH * N)
        .astype(ml_dtypes.bfloat16)
    )

    # cs[w, jj, h, jh, i] = round(184.66*(bias[h,i,j] + mask[w,i,j]) + C0), u16
    bias = table[index].reshape(N, N, NH)  # [i, j, h]
    biasT = np.ascontiguousarray(bias.transpose(2, 1, 0))  # [h, j, i]
    maskT = mask.transpose(0, 2, 1)  # [w, j, i]
    cfull = np.rint(
        (biasT[None] + maskT[:, None]) * np.float32(128.0 * LOG2E)
        + np.float32(SCHRAUD_C0)
    ).astype(np.uint16)
    cfull = np.ascontiguousarray(
        cfull.reshape(NW, NH, 2, 128, N).transpose(0, 3, 1, 2, 4)
    ).reshape(NW, 128, 2 * NH * N)

    # vp[jh, jj, l, h*33+c]
    v_aug = np.concatenate([v, np.ones((B_, NH, N, 1), np.float32)], axis=-1)

    in_maps = []
    bg_lists = []
    for c in range(NCORES):
        bg = np.array(
            [b * NW + 8 * c + wl for wl in range(NWC) for b in range(NBATCH)]
        )
        bg_lists.append(bg)
        va = v_aug[bg]  # [32, NH, N, 33]
        vpc = np.ascontiguousarray(
            va.transpose(2, 0, 1, 3)
            .reshape(2, 128, NB, NH * VC)
            .transpose(1, 0, 2, 3)
            .reshape(128, 2 * NB * NH * VC)
        ).astype(ml_dtypes.bfloat16)
        in_maps.append(
            {
                "ab": np.ascontiguousarray(ab_full[bg]),
                "cs": np.ascontiguousarray(cfull[8 * c : 8 * c + 8]),
                "vp": vpc,
            }
        )
    return in_maps, bg_lists


def postprocess(po_arr):
    """Normalize one core's pv output: [NB, 128, 2*NH*VC] -> [NB, N, NH*HD]."""
    v = np.asarray(po_arr, np.float32).reshape(NB, 128, 2, NH, VC)
    x = v[..., :HD] / v[..., HD:HD + 1]        # [NB, p, ih, NH, HD]
    return np.ascontiguousarray(x.transpose(0, 2, 1, 3, 4)).reshape(NB, N, NH * HD)


_NC_CACHE = {}


def get_nc():
    if "nc" not in _NC_CACHE:
        _NC_CACHE["nc"] = build_nc()
    return _NC_CACHE["nc"]


def kernel(q, k, v, table, mask, index):
    from concourse.bass_utils import run_bass_kernel_spmd

    in_maps, bg_lists = prep_inputs(q, k, v, table, mask, index)
    nc = get_nc()
    res = run_bass_kernel_spmd(nc, in_maps, core_ids=list(range(NCORES)))
    out = np.empty((B_, N, NH * HD), np.float32)
    for c in range(NCORES):
        out[bg_lists[c]] = postprocess(res.results[c]["po"])
    return out


if __name__ == "__main__":
    nc = build_nc()
    print("build + compile OK")
